# revision 8
# baseline (speedup 1.0000x reference)
"""LightGCN on 8 Trainium2 NeuronCores (Bass/Tile).

Scheme (src-sharded, ReduceScatter), v4:
- Node table padded to N_PAD = 8*18816 rows, stored per core as a bf16
  [WIN, 128] window table (cols 0:64 = features, 64:128 pad so rows are
  256B for dma_gather).
- Edge (src, dst, val) is processed by core src // WIN. Per core, edges
  are bucketed into 1176 dst-blocks of 128 dsts laid out chunk-major in a
  slot stream; block b occupies ceil128(max_core(count(c,b))) slots.
  (Sub-128-K matmul fragments at base partitions 32/64 crash on HW, so
  blocks stay 128-quantized; the frag machinery still supports packing.)
- Layer 3 frontier sparsification: gamma only reads acc rows for the
  batch's distinct users/items (F, ~29K rows), so layer 3 processes only
  edges with dst in F (~19% of edges), cutting its gather descriptor
  load (the bottleneck: ~9ns/row on the serial GpSimd Q7) by ~70%.
  Non-F output rows become zero, which is harmless in acc.
- Per call of NB tiles: dma_gather x[src] rows (bf16), build
  S[slot, dstcol] = val * (scol == iota) with 2 DVE ops, TensorEngine
  matmul fragments S^T @ X accumulate each block's [128, 64] partial sums
  in PSUM.
- Partials go bf16 to a permuted DRAM table P' so a chunked bf16
  ReduceScatter (3 chunks of 49*128 rows per window) hands each core its
  window's new embeddings at half the collective traffic (the RS/SDMA
  contention otherwise stalls gather descriptor DMA). acc accumulates in
  fp32 in SBUF; final AllGather + windowed gathers of user/item rows +
  dot products produce gamma.
"""
import os
import numpy as np
import ml_dtypes

BF = ml_dtypes.bfloat16
LAST_RESULTS = None  # BassKernelResults of the most recent run (for test.py)

N_USERS = 100000
N_ITEMS = 50000
N = N_USERS + N_ITEMS
D = 64
NC = 8
WIN = 18816             # 147 * 128 nodes per core window
N_PAD = NC * WIN        # 150528
NBLOCKS = N_PAD // 128  # 1176
NCHUNK = 7              # must divide 147 so QROWS is a tile multiple
QROWS = WIN // NCHUNK   # 2688 = 21*128
CHROWS = NC * QROWS     # 21504 rows per P' chunk
BPC = NBLOCKS // NCHUNK # 168 blocks per chunk (multiple of 8)
ROWB = 128              # table row elems (256B)
NB = 48                 # tiles per gather call
CALL = NB * 128         # slots per call
N_LAYERS = 3


def _chunk_of_block(b):
    return ((b * 128) % WIN) // QROWS


def _block_order():
    return sorted(range(NBLOCKS), key=lambda b: (_chunk_of_block(b), b))


def _prep_layer(src, dst, val, dst_mask=None):
    """Build the packed slot stream for one layer.

    Returns (sched, per_core, S_pad) where sched = [(block, m_b, off_b)]
    in chunk-major order with m_b = max over cores of the block's edge
    count rounded up to 32 (PE base-partition granularity), offsets
    nudged so no block starts at partition 96 (illegal matmul base), and
    per_core = [(idx16, scol, val)] packed streams of length S_pad.
    """
    if dst_mask is not None:
        sel = dst_mask[dst]
        src, dst, val = src[sel], dst[sel], val[sel]
    core = src // WIN
    blk = dst // 128

    counts = np.zeros((NC, NBLOCKS), dtype=np.int64)
    np.add.at(counts, (core, blk), 1)
    # 128-quantized block sizes: every matmul fragment is a full-K tile
    # (base partition 0), the only shape proven safe on HW.
    m = (np.maximum(1, counts.max(axis=0)) + 127) // 128 * 128

    order = _block_order()
    sched = []
    offs = {}
    off = 0
    for b in order:
        sched.append((b, int(m[b]), off))
        offs[b] = off
        off += int(m[b])
    S_raw = off
    S_pad = -(-S_raw // CALL) * CALL

    ordk = np.lexsort((dst, blk, core))
    src_s, dst_s, val_s, core_s, blk_s = (
        src[ordk], dst[ordk], val[ordk], core[ordk], blk[ordk])
    key = core_s * NBLOCKS + blk_s
    per_core = []
    for c in range(NC):
        idx16 = np.zeros(S_pad, dtype=np.int16)
        scol = np.zeros(S_pad, dtype=np.float32)
        v = np.zeros(S_pad, dtype=np.float32)
        lo = np.searchsorted(key, c * NBLOCKS)
        hi = np.searchsorted(key, (c + 1) * NBLOCKS)
        bsrc, bdst, bval, bblk = (
            src_s[lo:hi], dst_s[lo:hi], val_s[lo:hi], blk_s[lo:hi])
        bounds = np.searchsorted(bblk, np.arange(NBLOCKS + 1))
        for b in range(NBLOCKS):
            a, e = bounds[b], bounds[b + 1]
            if e == a:
                continue
            o = offs[b]
            n = e - a
            idx16[o:o + n] = (bsrc[a:e] - c * WIN).astype(np.int16)
            scol[o:o + n] = (bdst[a:e] - b * 128).astype(np.float32)
            v[o:o + n] = bval[a:e]
        per_core.append((idx16, scol, v))
    return sched, per_core, S_pad


def _legal_ranges(p0, p1):
    """Split a partition range into PE-legal (base, end) pieces.

    Matmul base partition must be in {0, 32, 64}; K <= 32 at base 32 and
    K <= 64 at base 64. Block offsets are 32-aligned and never 96.
    """
    if p0 == 0 or p0 == 64:
        return [(p0, p1)]
    assert p0 == 32
    if p1 <= 64:
        return [(32, p1)]
    return [(32, 64), (64, p1)]


def _layer_frags(sched, S_pad):
    """Per-call matmul fragments.

    frag = (tile, p0, p1, bpos, first, last). Tiles are 128-slot windows
    of the packed stream; a block's slots [off, off+m) may span several
    tiles and share boundary tiles with neighbors.
    """
    n_calls = S_pad // CALL
    frags_by_call = [[] for _ in range(n_calls)]
    for bpos, (b, mb, off) in enumerate(sched):
        lo, hi = off, off + mb
        t0, t1 = lo // 128, (hi - 1) // 128
        pieces = []
        for t in range(t0, t1 + 1):
            p0 = max(0, lo - t * 128)
            p1 = min(128, hi - t * 128)
            for q0, q1 in _legal_ranges(p0, p1):
                pieces.append((t, q0, q1))
        for i, (t, q0, q1) in enumerate(pieces):
            frags_by_call[t // NB].append(
                (t % NB, q0, q1, bpos, i == 0, i == len(pieces) - 1))
    return frags_by_call


def _wrap_idx(idx_flat):
    S = idx_flat.shape[0]
    w = idx_flat.reshape(S // 16, 16).T
    return np.tile(w, (8, 1)).copy()


def _wrap_slots(a_flat):
    S = a_flat.shape[0]
    return a_flat.reshape(S // 128, 128).T.copy()


def _prep_final(users, items):
    B = users.shape[0]
    pcn = B // NC
    u = users.astype(np.int64)
    it = items.astype(np.int64) + N_USERS
    uw = u // WIN
    iw = it // WIN
    combos = sorted({(int(a), int(b)) for a, b in zip(uw, iw)})
    gmax = {}
    for c in range(NC):
        lo, hi = c * pcn, (c + 1) * pcn
        for cu, ci in combos:
            mm = int(((uw[lo:hi] == cu) & (iw[lo:hi] == ci)).sum())
            gmax[(cu, ci)] = max(gmax.get((cu, ci), 0), mm)
    combos_sizes = [(k, (gmax[k] + 127) // 128 * 128) for k in combos]
    pad_total = sum(s for _, s in combos_sizes)
    plans = []
    for c in range(NC):
        lo, hi = c * pcn, (c + 1) * pcn
        slots_pair = np.full(pad_total, -1, dtype=np.int64)
        uidx = np.zeros(pad_total, dtype=np.int16)
        iidx = np.zeros(pad_total, dtype=np.int16)
        usub, isub = [], []
        off = 0
        for (cu, ci), size in combos_sizes:
            sel = np.nonzero((uw[lo:hi] == cu) & (iw[lo:hi] == ci))[0]
            n = sel.shape[0]
            slots_pair[off:off + n] = lo + sel
            uidx[off:off + n] = (u[lo + sel] - cu * WIN).astype(np.int16)
            iidx[off:off + n] = (it[lo + sel] - ci * WIN).astype(np.int16)
            usub.append((cu, off, off + size))
            isub.append((ci, off, off + size))
            off += size
        plans.append({"slots_pair": slots_pair, "uidx": uidx, "iidx": iidx,
                      "usub": usub, "isub": isub, "ntot": pad_total})
    return plans, combos_sizes, pad_total


def _build_program(scheds, S_pads, fin_subs, FS):
    """Build + compile the shared 8-core program.

    scheds/S_pads: per-layer schedules (layer 0 and 1 share stream
    tensors; layer 2 has its own frontier-restricted stream).
    """
    import concourse.bacc as bacc
    import concourse.tile as tile
    from concourse import mybir

    FT = FS // 128
    nc = bacc.Bacc("TRN2", target_bir_lowering=False, debug=False,
                   num_devices=NC)
    dt = mybir.dt

    t0_in = nc.dram_tensor("t0", [WIN, ROWB], dt.bfloat16, kind="ExternalInput")
    x0_in = nc.dram_tensor("x0", [WIN, D], dt.float32, kind="ExternalInput")
    T01 = S_pads[0] // 128
    T2 = S_pads[2] // 128
    idx01_in = nc.dram_tensor("idx01", [128, T01 * 8], dt.int16,
                              kind="ExternalInput")
    scol01_in = nc.dram_tensor("scol01", [128, T01], dt.bfloat16,
                               kind="ExternalInput")
    val01_in = nc.dram_tensor("val01", [128, T01], dt.bfloat16,
                              kind="ExternalInput")
    idx2_in = nc.dram_tensor("idx2", [128, T2 * 8], dt.int16,
                             kind="ExternalInput")
    scol2_in = nc.dram_tensor("scol2", [128, T2], dt.bfloat16,
                              kind="ExternalInput")
    val2_in = nc.dram_tensor("val2", [128, T2], dt.bfloat16,
                             kind="ExternalInput")
    iota_in = nc.dram_tensor("iota", [128, 128], dt.bfloat16,
                             kind="ExternalInput")
    fu_in = nc.dram_tensor("fuidx", [128, FS // 16], dt.int16,
                           kind="ExternalInput")
    fi_in = nc.dram_tensor("fiidx", [128, FS // 16], dt.int16,
                           kind="ExternalInput")
    gamma_out = nc.dram_tensor("gamma", [128, FT], dt.float32,
                               kind="ExternalOutput")

    tp = [nc.dram_tensor(f"tbl{i}", [WIN, ROWB], dt.bfloat16, kind="Internal")
          for i in range(2)]
    read_t = [t0_in, tp[0], tp[1]]
    write_t = [tp[0], tp[1], None]
    lay_in = [(idx01_in, scol01_in, val01_in),
              (idx01_in, scol01_in, val01_in),
              (idx2_in, scol2_in, val2_in)]
    A_tab = nc.dram_tensor("atab", [N_PAD, D], dt.float32, kind="Internal")
    ag_in = nc.dram_tensor("agin", [WIN, D], dt.float32, kind="Internal")
    pp = [[nc.dram_tensor(f"pp_l{l}_q{q}", [CHROWS, D], dt.bfloat16,
                          kind="Internal") for q in range(NCHUNK)]
          for l in range(N_LAYERS)]
    rs = [[nc.dram_tensor(f"rs_l{l}_q{q}", [QROWS, D], dt.bfloat16,
                          kind="Internal")
           for q in range(NCHUNK)] for l in range(N_LAYERS)]

    frags_by_layer = [_layer_frags(scheds[l], S_pads[l])
                      for l in range(N_LAYERS)]

    with tile.TileContext(nc) as tc:
        with tc.tile_pool(name="persist", bufs=1) as pers, \
             tc.tile_pool(name="gbuf", bufs=3) as gpool, \
             tc.tile_pool(name="sbuf2", bufs=2) as spool, \
             tc.tile_pool(name="psum", bufs=3, space="PSUM") as ppool:

            iota_t = pers.tile([128, 128], dt.bfloat16)
            nc.sync.dma_start(iota_t[:], iota_in[:])
            acc_t = pers.tile([128, NCHUNK, QROWS // 128, D], dt.float32)
            nc.sync.dma_start(
                acc_t[:],
                x0_in.ap().rearrange("(q p t) d -> p q t d",
                                     q=NCHUNK, p=128, t=QROWS // 128))

            for layer in range(N_LAYERS):
                tbl = read_t[layer]
                idx_in, scol_in, val_in = lay_in[layer]
                frags_by_call = frags_by_layer[layer]
                n_calls = S_pads[layer] // CALL
                psum_t = None
                stage_t = None
                done = 0          # completed blocks this layer
                for ci in range(n_calls):
                    idx_t = gpool.tile([128, NB * 8], dt.int16, tag="idx")
                    nc.sync.dma_start(
                        idx_t[:], idx_in[:, ci * NB * 8:(ci + 1) * NB * 8])
                    scol_t = gpool.tile([128, NB], dt.bfloat16, tag="scol")
                    nc.sync.dma_start(
                        scol_t[:], scol_in[:, ci * NB:(ci + 1) * NB])
                    val_t = gpool.tile([128, NB], dt.bfloat16, tag="val")
                    nc.sync.dma_start(
                        val_t[:], val_in[:, ci * NB:(ci + 1) * NB])
                    g_t = gpool.tile([128, NB, ROWB], dt.bfloat16, tag="g")
                    # ucode limit: 1024 idxs per dma_gather call
                    for k in range(NB // 8):
                        nc.gpsimd.dma_gather(
                            out_ap=g_t[:, k * 8:(k + 1) * 8, :],
                            in_ap=tbl[:],
                            idxs_ap=idx_t[:, k * 64:(k + 1) * 64],
                            num_idxs=1024, num_idxs_reg=1024,
                            elem_size=ROWB)
                    seq_t = spool.tile([128, NB, 128], dt.bfloat16, tag="seq")
                    nc.vector.tensor_tensor(
                        seq_t[:],
                        scol_t[:].unsqueeze(-1).broadcast_to([128, NB, 128]),
                        iota_t[:].unsqueeze(1).broadcast_to([128, NB, 128]),
                        mybir.AluOpType.is_equal)
                    s_t = spool.tile([128, NB, 128], dt.bfloat16, tag="s")
                    nc.vector.tensor_tensor(
                        s_t[:],
                        seq_t[:],
                        val_t[:].unsqueeze(-1).broadcast_to([128, NB, 128]),
                        mybir.AluOpType.mult)

                    for (t, p0, p1, bpos, first, last) in frags_by_call[ci]:
                        if first and bpos % 4 == 0:
                            psum_t = ppool.tile([128, 4, D], dt.float32)
                        nc.tensor.matmul(
                            psum_t[:, bpos % 4, :],
                            s_t[p0:p1, t, :], g_t[p0:p1, t, 0:D],
                            start=first, stop=last)
                        if not last:
                            continue
                        # block bpos complete
                        if bpos % 8 == 0:
                            stage_t = spool.tile([128, 8, D], dt.bfloat16,
                                                 tag="stage")
                        if bpos % 4 == 3:
                            nc.scalar.activation(
                                stage_t[:, (bpos % 8) - 3:(bpos % 8) + 1, :],
                                psum_t[:],
                                mybir.ActivationFunctionType.Copy)
                        if bpos % 8 == 7:
                            q = bpos // BPC
                            r0 = (bpos % BPC - 7) * 128
                            nc.sync.dma_start(
                                pp[layer][q][r0:r0 + 1024, :].rearrange(
                                    "(j p) d -> p j d", p=128, j=8),
                                stage_t[:])
                        done += 1
                        if done % BPC == 0:
                            # chunk q done
                            q = done // BPC - 1
                            nc.gpsimd.collective_compute(
                                "ReduceScatter", mybir.AluOpType.add,
                                replica_groups=[list(range(NC))],
                                ins=[pp[layer][q].ap()],
                                outs=[rs[layer][q].ap()])
                            rsb = spool.tile([128, QROWS // 128, D],
                                             dt.bfloat16, tag="rsb")
                            nc.sync.dma_start(
                                rsb[:],
                                rs[layer][q].ap().rearrange(
                                    "(p t) d -> p t d", p=128,
                                    t=QROWS // 128))
                            if write_t[layer] is not None:
                                nc.sync.dma_start(
                                    write_t[layer][q * QROWS:(q + 1) * QROWS,
                                                   0:D].rearrange(
                                        "(p t) d -> p t d", p=128,
                                        t=QROWS // 128),
                                    rsb[:])
                            nc.vector.tensor_tensor(
                                acc_t[:, q, :, :], acc_t[:, q, :, :], rsb[:],
                                mybir.AluOpType.add)

            # final: light = acc/4 -> AllGather -> windowed gathers -> dots
            nc.vector.tensor_scalar_mul(acc_t[:], acc_t[:], 0.25)
            nc.sync.dma_start(
                ag_in.ap().rearrange("(q p t) d -> p q t d",
                                     q=NCHUNK, p=128, t=QROWS // 128),
                acc_t[:])
            nc.gpsimd.collective_compute(
                "AllGather", mybir.AluOpType.bypass,
                replica_groups=[list(range(NC))],
                ins=[ag_in.ap()], outs=[A_tab.ap()])

            fu_t = pers.tile([128, FS // 16], dt.int16)
            nc.sync.dma_start(fu_t[:], fu_in[:])
            fi_t = pers.tile([128, FS // 16], dt.int16)
            nc.sync.dma_start(fi_t[:], fi_in[:])
            ug_t = pers.tile([128, FT, D], dt.float32)
            ig_t = pers.tile([128, FT, D], dt.float32)
            for subs, idxt, outt in ((fin_subs[0], fu_t, ug_t),
                                     (fin_subs[1], fi_t, ig_t)):
                for w, lo, hi in subs:
                    for a in range(lo, hi, 1024):
                        n = min(1024, hi - a)
                        nc.gpsimd.dma_gather(
                            out_ap=outt[:, a // 128:(a + n) // 128, :],
                            in_ap=A_tab[w * WIN:(w + 1) * WIN, :],
                            idxs_ap=idxt[:, a // 16:(a + n) // 16],
                            num_idxs=n, num_idxs_reg=n,
                            elem_size=D)
            prod_t = pers.tile([128, FT, D], dt.float32)
            nc.vector.tensor_mul(prod_t[:], ug_t[:], ig_t[:])
            gam_t = pers.tile([128, FT], dt.float32)
            nc.vector.tensor_reduce(
                gam_t[:].unsqueeze(-1), prod_t[:],
                axis=mybir.AxisListType.X, op=mybir.AluOpType.add)
            nc.sync.dma_start(gamma_out[:], gam_t[:])

    nc.compile()
    return nc


def kernel(**inputs):
    from concourse import bass_utils

    users = np.asarray(inputs["users"])
    items = np.asarray(inputs["items"])
    edge_src = np.asarray(inputs["edge_src"]).astype(np.int64)
    edge_dst = np.asarray(inputs["edge_dst"]).astype(np.int64)
    edge_val = np.asarray(inputs["edge_val"], dtype=np.float32)
    user_emb = np.asarray(inputs["user_emb"], dtype=np.float32)
    item_emb = np.asarray(inputs["item_emb"], dtype=np.float32)

    F = np.union1d(np.unique(users.astype(np.int64)),
                   np.unique(items.astype(np.int64) + N_USERS))
    fmask = np.zeros(N_PAD, dtype=bool)
    fmask[F] = True

    sched01, per_core01, S01 = _prep_layer(edge_src, edge_dst, edge_val)
    sched2, per_core2, S2 = _prep_layer(edge_src, edge_dst, edge_val,
                                        dst_mask=fmask)
    scheds = [sched01, sched01, sched2]
    S_pads = [S01, S01, S2]

    plans, combos_sizes, FS = _prep_final(users, items)
    fin_subs = (plans[0]["usub"], plans[0]["isub"])

    nc = _build_program(scheds, S_pads, fin_subs, FS)

    x0 = np.zeros((N_PAD, D), dtype=np.float32)
    x0[:N_USERS] = user_emb
    x0[N_USERS:N] = item_emb
    iota_img = np.tile(np.arange(128, dtype=np.float32).astype(BF)[None, :],
                       (128, 1)).copy()

    in_maps = []
    for c in range(NC):
        idx01, scol01, val01 = per_core01[c]
        idx2, scol2, val2 = per_core2[c]
        t0 = np.zeros((WIN, ROWB), dtype=BF)
        t0[:, :D] = x0[c * WIN:(c + 1) * WIN].astype(BF)
        pl = plans[c]
        in_maps.append({
            "t0": t0,
            "x0": x0[c * WIN:(c + 1) * WIN].copy(),
            "idx01": _wrap_idx(idx01),
            "scol01": _wrap_slots(scol01.astype(BF)),
            "val01": _wrap_slots(val01.astype(BF)),
            "idx2": _wrap_idx(idx2),
            "scol2": _wrap_slots(scol2.astype(BF)),
            "val2": _wrap_slots(val2.astype(BF)),
            "iota": iota_img,
            "fuidx": _wrap_idx(pl["uidx"]),
            "fiidx": _wrap_idx(pl["iidx"]),
        })

    res = bass_utils.run_bass_kernel_spmd(
        nc, in_maps, core_ids=list(range(NC)),
        trace=bool(os.environ.get("KERNEL_TRACE")))
    global LAST_RESULTS
    LAST_RESULTS = res

    gamma = np.zeros(users.shape[0], dtype=np.float32)
    for c in range(NC):
        img = res.results[c]["gamma"]          # [128, FT]
        flat = img.T.reshape(-1)               # slot s = 128*t + p
        pl = plans[c]
        sel = pl["slots_pair"] >= 0
        gamma[pl["slots_pair"][sel]] = flat[sel]
    return gamma


# revision 9
# speedup vs baseline: 1.0266x; 1.0266x over previous
"""LightGCN on 8 Trainium2 NeuronCores (Bass/Tile).

Scheme (src-sharded, ReduceScatter), v4:
- Node table padded to N_PAD = 8*18816 rows, stored per core as a bf16
  [WIN, 128] window table (cols 0:64 = features, 64:128 pad so rows are
  256B for dma_gather).
- Edge (src, dst, val) is processed by core src // WIN. Per core, edges
  are bucketed into 1176 dst-blocks of 128 dsts laid out chunk-major in a
  slot stream; block b occupies ceil128(max_core(count(c,b))) slots.
  (Sub-128-K matmul fragments at base partitions 32/64 crash on HW, so
  blocks stay 128-quantized; the frag machinery still supports packing.)
- Layer 3 frontier sparsification: gamma only reads acc rows for the
  batch's distinct users/items (F, ~29K rows), so layer 3 processes only
  edges with dst in F (~19% of edges), cutting its gather descriptor
  load (the bottleneck: ~9ns/row on the serial GpSimd Q7) by ~70%.
  Non-F output rows become zero, which is harmless in acc.
- Per call of NB tiles: dma_gather x[src] rows (bf16), build
  S[slot, dstcol] = val * (scol == iota) with 2 DVE ops, TensorEngine
  matmul fragments S^T @ X accumulate each block's [128, 64] partial sums
  in PSUM.
- Partials go bf16 to a permuted DRAM table P' so a chunked bf16
  ReduceScatter (3 chunks of 49*128 rows per window) hands each core its
  window's new embeddings at half the collective traffic (the RS/SDMA
  contention otherwise stalls gather descriptor DMA). acc accumulates in
  fp32 in SBUF; final AllGather + windowed gathers of user/item rows +
  dot products produce gamma.
"""
import os
import numpy as np
import ml_dtypes

BF = ml_dtypes.bfloat16
LAST_RESULTS = None  # BassKernelResults of the most recent run (for test.py)

N_USERS = 100000
N_ITEMS = 50000
N = N_USERS + N_ITEMS
D = 64
NC = 8
WIN = 18816             # 147 * 128 nodes per core window
N_PAD = NC * WIN        # 150528
NBLOCKS = N_PAD // 128  # 1176
NCHUNK = 7              # must divide 147 so QROWS is a tile multiple
QROWS = WIN // NCHUNK   # 2688 = 21*128
CHROWS = NC * QROWS     # 21504 rows per P' chunk
BPC = NBLOCKS // NCHUNK # 168 blocks per chunk (multiple of 8)
ROWB = 128              # table row elems (256B)
NB = 48                 # tiles per gather call
CALL = NB * 128         # slots per call
N_LAYERS = 3


def _chunk_of_block(b):
    return ((b * 128) % WIN) // QROWS


def _block_order():
    return sorted(range(NBLOCKS), key=lambda b: (_chunk_of_block(b), b))


def _prep_layer(src, dst, val, dst_mask=None):
    """Build the packed slot stream for one layer.

    Returns (sched, per_core, S_pad) where sched = [(block, m_b, off_b)]
    in chunk-major order with m_b = max over cores of the block's edge
    count rounded up to 32 (PE base-partition granularity), offsets
    nudged so no block starts at partition 96 (illegal matmul base), and
    per_core = [(idx16, scol, val)] packed streams of length S_pad.
    """
    if dst_mask is not None:
        sel = dst_mask[dst]
        src, dst, val = src[sel], dst[sel], val[sel]
    core = src // WIN
    blk = dst // 128

    counts = np.zeros((NC, NBLOCKS), dtype=np.int64)
    np.add.at(counts, (core, blk), 1)
    # 128-quantized block sizes: every matmul fragment is a full-K tile
    # (base partition 0), the only shape proven safe on HW.
    m = (np.maximum(1, counts.max(axis=0)) + 127) // 128 * 128

    order = _block_order()
    sched = []
    offs = {}
    off = 0
    for b in order:
        sched.append((b, int(m[b]), off))
        offs[b] = off
        off += int(m[b])
    S_raw = off
    S_pad = -(-S_raw // CALL) * CALL

    ordk = np.lexsort((dst, blk, core))
    src_s, dst_s, val_s, core_s, blk_s = (
        src[ordk], dst[ordk], val[ordk], core[ordk], blk[ordk])
    key = core_s * NBLOCKS + blk_s
    per_core = []
    for c in range(NC):
        idx16 = np.zeros(S_pad, dtype=np.int16)
        scol = np.zeros(S_pad, dtype=np.float32)
        v = np.zeros(S_pad, dtype=np.float32)
        lo = np.searchsorted(key, c * NBLOCKS)
        hi = np.searchsorted(key, (c + 1) * NBLOCKS)
        bsrc, bdst, bval, bblk = (
            src_s[lo:hi], dst_s[lo:hi], val_s[lo:hi], blk_s[lo:hi])
        bounds = np.searchsorted(bblk, np.arange(NBLOCKS + 1))
        for b in range(NBLOCKS):
            a, e = bounds[b], bounds[b + 1]
            if e == a:
                continue
            o = offs[b]
            n = e - a
            idx16[o:o + n] = (bsrc[a:e] - c * WIN).astype(np.int16)
            scol[o:o + n] = (bdst[a:e] - b * 128).astype(np.float32)
            v[o:o + n] = bval[a:e]
        per_core.append((idx16, scol, v))
    return sched, per_core, S_pad


def _legal_ranges(p0, p1):
    """Split a partition range into PE-legal (base, end) pieces.

    Matmul base partition must be in {0, 32, 64}; K <= 32 at base 32 and
    K <= 64 at base 64. Block offsets are 32-aligned and never 96.
    """
    if p0 == 0 or p0 == 64:
        return [(p0, p1)]
    assert p0 == 32
    if p1 <= 64:
        return [(32, p1)]
    return [(32, 64), (64, p1)]


def _layer_frags(sched, S_pad):
    """Per-call matmul fragments.

    frag = (tile, p0, p1, bpos, first, last). Tiles are 128-slot windows
    of the packed stream; a block's slots [off, off+m) may span several
    tiles and share boundary tiles with neighbors.
    """
    n_calls = S_pad // CALL
    frags_by_call = [[] for _ in range(n_calls)]
    for bpos, (b, mb, off) in enumerate(sched):
        lo, hi = off, off + mb
        t0, t1 = lo // 128, (hi - 1) // 128
        pieces = []
        for t in range(t0, t1 + 1):
            p0 = max(0, lo - t * 128)
            p1 = min(128, hi - t * 128)
            for q0, q1 in _legal_ranges(p0, p1):
                pieces.append((t, q0, q1))
        for i, (t, q0, q1) in enumerate(pieces):
            frags_by_call[t // NB].append(
                (t % NB, q0, q1, bpos, i == 0, i == len(pieces) - 1))
    return frags_by_call


def _wrap_idx(idx_flat):
    S = idx_flat.shape[0]
    w = idx_flat.reshape(S // 16, 16).T
    return np.tile(w, (8, 1)).copy()


def _wrap_slots(a_flat):
    S = a_flat.shape[0]
    return a_flat.reshape(S // 128, 128).T.copy()


def _prep_final(users, items):
    B = users.shape[0]
    pcn = B // NC
    u = users.astype(np.int64)
    it = items.astype(np.int64) + N_USERS
    uw = u // WIN
    iw = it // WIN
    combos = sorted({(int(a), int(b)) for a, b in zip(uw, iw)})
    gmax = {}
    for c in range(NC):
        lo, hi = c * pcn, (c + 1) * pcn
        for cu, ci in combos:
            mm = int(((uw[lo:hi] == cu) & (iw[lo:hi] == ci)).sum())
            gmax[(cu, ci)] = max(gmax.get((cu, ci), 0), mm)
    combos_sizes = [(k, (gmax[k] + 127) // 128 * 128) for k in combos]
    pad_total = sum(s for _, s in combos_sizes)
    plans = []
    for c in range(NC):
        lo, hi = c * pcn, (c + 1) * pcn
        slots_pair = np.full(pad_total, -1, dtype=np.int64)
        uidx = np.zeros(pad_total, dtype=np.int16)
        iidx = np.zeros(pad_total, dtype=np.int16)
        usub, isub = [], []
        off = 0
        for (cu, ci), size in combos_sizes:
            sel = np.nonzero((uw[lo:hi] == cu) & (iw[lo:hi] == ci))[0]
            n = sel.shape[0]
            slots_pair[off:off + n] = lo + sel
            uidx[off:off + n] = (u[lo + sel] - cu * WIN).astype(np.int16)
            iidx[off:off + n] = (it[lo + sel] - ci * WIN).astype(np.int16)
            usub.append((cu, off, off + size))
            isub.append((ci, off, off + size))
            off += size
        plans.append({"slots_pair": slots_pair, "uidx": uidx, "iidx": iidx,
                      "usub": usub, "isub": isub, "ntot": pad_total})
    return plans, combos_sizes, pad_total


def _build_program(scheds, S_pads, fin_subs, FS):
    """Build + compile the shared 8-core program.

    scheds/S_pads: per-layer schedules (layer 0 and 1 share stream
    tensors; layer 2 has its own frontier-restricted stream).
    """
    import concourse.bacc as bacc
    import concourse.tile as tile
    from concourse import mybir

    FT = FS // 128
    nc = bacc.Bacc("TRN2", target_bir_lowering=False, debug=False,
                   num_devices=NC)
    dt = mybir.dt

    t0_in = nc.dram_tensor("t0", [WIN, ROWB], dt.bfloat16, kind="ExternalInput")
    x0_in = nc.dram_tensor("x0", [WIN, D], dt.float32, kind="ExternalInput")
    T01 = S_pads[0] // 128
    T2 = S_pads[2] // 128
    idx01_in = nc.dram_tensor("idx01", [128, T01 * 8], dt.int16,
                              kind="ExternalInput")
    scol01_in = nc.dram_tensor("scol01", [128, T01], dt.bfloat16,
                               kind="ExternalInput")
    val01_in = nc.dram_tensor("val01", [128, T01], dt.bfloat16,
                              kind="ExternalInput")
    idx2_in = nc.dram_tensor("idx2", [128, T2 * 8], dt.int16,
                             kind="ExternalInput")
    scol2_in = nc.dram_tensor("scol2", [128, T2], dt.bfloat16,
                              kind="ExternalInput")
    val2_in = nc.dram_tensor("val2", [128, T2], dt.bfloat16,
                             kind="ExternalInput")
    iota_in = nc.dram_tensor("iota", [128, 128], dt.bfloat16,
                             kind="ExternalInput")
    fu_in = nc.dram_tensor("fuidx", [128, FS // 16], dt.int16,
                           kind="ExternalInput")
    fi_in = nc.dram_tensor("fiidx", [128, FS // 16], dt.int16,
                           kind="ExternalInput")
    gamma_out = nc.dram_tensor("gamma", [128, FT], dt.float32,
                               kind="ExternalOutput")

    tp = [nc.dram_tensor(f"tbl{i}", [WIN, ROWB], dt.bfloat16, kind="Internal")
          for i in range(2)]
    read_t = [t0_in, tp[0], tp[1]]
    write_t = [tp[0], tp[1], None]
    lay_in = [(idx01_in, scol01_in, val01_in),
              (idx01_in, scol01_in, val01_in),
              (idx2_in, scol2_in, val2_in)]
    A_tab = nc.dram_tensor("atab", [N_PAD, D], dt.float32, kind="Internal")
    ag_in = nc.dram_tensor("agin", [WIN, D], dt.float32, kind="Internal")
    pp = [[nc.dram_tensor(f"pp_l{l}_q{q}", [CHROWS, D], dt.bfloat16,
                          kind="Internal") for q in range(NCHUNK)]
          for l in range(N_LAYERS)]
    rs = [[nc.dram_tensor(f"rs_l{l}_q{q}", [QROWS, D], dt.bfloat16,
                          kind="Internal")
           for q in range(NCHUNK)] for l in range(N_LAYERS)]

    frags_by_layer = [_layer_frags(scheds[l], S_pads[l])
                      for l in range(N_LAYERS)]

    with tile.TileContext(nc) as tc:
        with tc.tile_pool(name="persist", bufs=1) as pers, \
             tc.tile_pool(name="inb", bufs=6) as ipool, \
             tc.tile_pool(name="gbuf", bufs=4) as gpool, \
             tc.tile_pool(name="sbuf2", bufs=2) as spool, \
             tc.tile_pool(name="psum", bufs=3, space="PSUM") as ppool:

            iota_t = pers.tile([128, 128], dt.bfloat16)
            nc.sync.dma_start(iota_t[:], iota_in[:])
            acc_t = pers.tile([128, NCHUNK, QROWS // 128, D], dt.float32)
            nc.sync.dma_start(
                acc_t[:],
                x0_in.ap().rearrange("(q p t) d -> p q t d",
                                     q=NCHUNK, p=128, t=QROWS // 128))
            fu_t = pers.tile([128, FS // 16], dt.int16)
            nc.sync.dma_start(fu_t[:], fu_in[:])
            fi_t = pers.tile([128, FS // 16], dt.int16)
            nc.sync.dma_start(fi_t[:], fi_in[:])

            for layer in range(N_LAYERS):
                tbl = read_t[layer]
                idx_in, scol_in, val_in = lay_in[layer]
                frags_by_call = frags_by_layer[layer]
                n_calls = S_pads[layer] // CALL
                psum_t = None
                stage_t = None
                done = 0          # completed blocks this layer
                for ci in range(n_calls):
                    idx_t = ipool.tile([128, NB * 8], dt.int16, tag="idx")
                    nc.sync.dma_start(
                        idx_t[:], idx_in[:, ci * NB * 8:(ci + 1) * NB * 8])
                    scol_t = ipool.tile([128, NB], dt.bfloat16, tag="scol")
                    nc.sync.dma_start(
                        scol_t[:], scol_in[:, ci * NB:(ci + 1) * NB])
                    val_t = ipool.tile([128, NB], dt.bfloat16, tag="val")
                    nc.sync.dma_start(
                        val_t[:], val_in[:, ci * NB:(ci + 1) * NB])
                    g_t = gpool.tile([128, NB, ROWB], dt.bfloat16, tag="g")
                    # ucode limit: 1024 idxs per dma_gather call
                    for k in range(NB // 8):
                        nc.gpsimd.dma_gather(
                            out_ap=g_t[:, k * 8:(k + 1) * 8, :],
                            in_ap=tbl[:],
                            idxs_ap=idx_t[:, k * 64:(k + 1) * 64],
                            num_idxs=1024, num_idxs_reg=1024,
                            elem_size=ROWB)
                    seq_t = spool.tile([128, NB, 128], dt.bfloat16, tag="seq")
                    nc.vector.tensor_tensor(
                        seq_t[:],
                        scol_t[:].unsqueeze(-1).broadcast_to([128, NB, 128]),
                        iota_t[:].unsqueeze(1).broadcast_to([128, NB, 128]),
                        mybir.AluOpType.is_equal)
                    s_t = spool.tile([128, NB, 128], dt.bfloat16, tag="s")
                    nc.vector.tensor_tensor(
                        s_t[:],
                        seq_t[:],
                        val_t[:].unsqueeze(-1).broadcast_to([128, NB, 128]),
                        mybir.AluOpType.mult)

                    for (t, p0, p1, bpos, first, last) in frags_by_call[ci]:
                        if first and bpos % 4 == 0:
                            psum_t = ppool.tile([128, 4, D], dt.float32)
                        nc.tensor.matmul(
                            psum_t[:, bpos % 4, :],
                            s_t[p0:p1, t, :], g_t[p0:p1, t, 0:D],
                            start=first, stop=last)
                        if not last:
                            continue
                        # block bpos complete
                        if bpos % 8 == 0:
                            stage_t = spool.tile([128, 8, D], dt.bfloat16,
                                                 tag="stage")
                        if bpos % 4 == 3:
                            nc.scalar.activation(
                                stage_t[:, (bpos % 8) - 3:(bpos % 8) + 1, :],
                                psum_t[:],
                                mybir.ActivationFunctionType.Copy)
                        if bpos % 8 == 7:
                            q = bpos // BPC
                            r0 = (bpos % BPC - 7) * 128
                            nc.sync.dma_start(
                                pp[layer][q][r0:r0 + 1024, :].rearrange(
                                    "(j p) d -> p j d", p=128, j=8),
                                stage_t[:])
                        done += 1
                        if done % BPC == 0:
                            # chunk q done
                            q = done // BPC - 1
                            nc.gpsimd.collective_compute(
                                "ReduceScatter", mybir.AluOpType.add,
                                replica_groups=[list(range(NC))],
                                ins=[pp[layer][q].ap()],
                                outs=[rs[layer][q].ap()])
                            rsb = spool.tile([128, QROWS // 128, D],
                                             dt.bfloat16, tag="rsb")
                            nc.sync.dma_start(
                                rsb[:],
                                rs[layer][q].ap().rearrange(
                                    "(p t) d -> p t d", p=128,
                                    t=QROWS // 128))
                            if write_t[layer] is not None:
                                nc.sync.dma_start(
                                    write_t[layer][q * QROWS:(q + 1) * QROWS,
                                                   0:D].rearrange(
                                        "(p t) d -> p t d", p=128,
                                        t=QROWS // 128),
                                    rsb[:])
                            nc.vector.tensor_tensor(
                                acc_t[:, q, :, :], acc_t[:, q, :, :], rsb[:],
                                mybir.AluOpType.add)

            # final: light = acc/4 -> AllGather -> windowed gathers -> dots
            nc.vector.tensor_scalar_mul(acc_t[:], acc_t[:], 0.25)
            nc.sync.dma_start(
                ag_in.ap().rearrange("(q p t) d -> p q t d",
                                     q=NCHUNK, p=128, t=QROWS // 128),
                acc_t[:])
            nc.gpsimd.collective_compute(
                "AllGather", mybir.AluOpType.bypass,
                replica_groups=[list(range(NC))],
                ins=[ag_in.ap()], outs=[A_tab.ap()])

            ug_t = pers.tile([128, FT, D], dt.float32)
            ig_t = pers.tile([128, FT, D], dt.float32)
            for subs, idxt, outt in ((fin_subs[0], fu_t, ug_t),
                                     (fin_subs[1], fi_t, ig_t)):
                for w, lo, hi in subs:
                    for a in range(lo, hi, 1024):
                        n = min(1024, hi - a)
                        nc.gpsimd.dma_gather(
                            out_ap=outt[:, a // 128:(a + n) // 128, :],
                            in_ap=A_tab[w * WIN:(w + 1) * WIN, :],
                            idxs_ap=idxt[:, a // 16:(a + n) // 16],
                            num_idxs=n, num_idxs_reg=n,
                            elem_size=D)
            prod_t = pers.tile([128, FT, D], dt.float32)
            nc.vector.tensor_mul(prod_t[:], ug_t[:], ig_t[:])
            gam_t = pers.tile([128, FT], dt.float32)
            nc.vector.tensor_reduce(
                gam_t[:].unsqueeze(-1), prod_t[:],
                axis=mybir.AxisListType.X, op=mybir.AluOpType.add)
            nc.sync.dma_start(gamma_out[:], gam_t[:])

    nc.compile()
    return nc


def kernel(**inputs):
    from concourse import bass_utils

    users = np.asarray(inputs["users"])
    items = np.asarray(inputs["items"])
    edge_src = np.asarray(inputs["edge_src"]).astype(np.int64)
    edge_dst = np.asarray(inputs["edge_dst"]).astype(np.int64)
    edge_val = np.asarray(inputs["edge_val"], dtype=np.float32)
    user_emb = np.asarray(inputs["user_emb"], dtype=np.float32)
    item_emb = np.asarray(inputs["item_emb"], dtype=np.float32)

    F = np.union1d(np.unique(users.astype(np.int64)),
                   np.unique(items.astype(np.int64) + N_USERS))
    fmask = np.zeros(N_PAD, dtype=bool)
    fmask[F] = True

    sched01, per_core01, S01 = _prep_layer(edge_src, edge_dst, edge_val)
    sched2, per_core2, S2 = _prep_layer(edge_src, edge_dst, edge_val,
                                        dst_mask=fmask)
    scheds = [sched01, sched01, sched2]
    S_pads = [S01, S01, S2]

    plans, combos_sizes, FS = _prep_final(users, items)
    fin_subs = (plans[0]["usub"], plans[0]["isub"])

    nc = _build_program(scheds, S_pads, fin_subs, FS)

    x0 = np.zeros((N_PAD, D), dtype=np.float32)
    x0[:N_USERS] = user_emb
    x0[N_USERS:N] = item_emb
    iota_img = np.tile(np.arange(128, dtype=np.float32).astype(BF)[None, :],
                       (128, 1)).copy()

    in_maps = []
    for c in range(NC):
        idx01, scol01, val01 = per_core01[c]
        idx2, scol2, val2 = per_core2[c]
        t0 = np.zeros((WIN, ROWB), dtype=BF)
        t0[:, :D] = x0[c * WIN:(c + 1) * WIN].astype(BF)
        pl = plans[c]
        in_maps.append({
            "t0": t0,
            "x0": x0[c * WIN:(c + 1) * WIN].copy(),
            "idx01": _wrap_idx(idx01),
            "scol01": _wrap_slots(scol01.astype(BF)),
            "val01": _wrap_slots(val01.astype(BF)),
            "idx2": _wrap_idx(idx2),
            "scol2": _wrap_slots(scol2.astype(BF)),
            "val2": _wrap_slots(val2.astype(BF)),
            "iota": iota_img,
            "fuidx": _wrap_idx(pl["uidx"]),
            "fiidx": _wrap_idx(pl["iidx"]),
        })

    res = bass_utils.run_bass_kernel_spmd(
        nc, in_maps, core_ids=list(range(NC)),
        trace=bool(os.environ.get("KERNEL_TRACE")))
    global LAST_RESULTS
    LAST_RESULTS = res

    gamma = np.zeros(users.shape[0], dtype=np.float32)
    for c in range(NC):
        img = res.results[c]["gamma"]          # [128, FT]
        flat = img.T.reshape(-1)               # slot s = 128*t + p
        pl = plans[c]
        sel = pl["slots_pair"] >= 0
        gamma[pl["slots_pair"][sel]] = flat[sel]
    return gamma


# revision 10
# speedup vs baseline: 1.0319x; 1.0051x over previous
"""LightGCN on 8 Trainium2 NeuronCores (Bass/Tile).

Scheme (src-sharded, ReduceScatter), v4:
- Node table padded to N_PAD = 8*18816 rows, stored per core as a bf16
  [WIN, 128] window table (cols 0:64 = features, 64:128 pad so rows are
  256B for dma_gather).
- Edge (src, dst, val) is processed by core src // WIN. Per core, edges
  are bucketed into 1176 dst-blocks of 128 dsts laid out chunk-major in a
  slot stream; block b occupies ceil128(max_core(count(c,b))) slots.
  (Sub-128-K matmul fragments at base partitions 32/64 crash on HW, so
  blocks stay 128-quantized; the frag machinery still supports packing.)
- Layer 3 frontier sparsification: gamma only reads acc rows for the
  batch's distinct users/items (F, ~29K rows), so layer 3 processes only
  edges with dst in F (~19% of edges), cutting its gather descriptor
  load (the bottleneck: ~9ns/row on the serial GpSimd Q7) by ~70%.
  Non-F output rows become zero, which is harmless in acc.
- Per call of NB tiles: dma_gather x[src] rows (bf16), build
  S[slot, dstcol] = val * (scol == iota) with 2 DVE ops, TensorEngine
  matmul fragments S^T @ X accumulate each block's [128, 64] partial sums
  in PSUM.
- Partials go bf16 to a permuted DRAM table P' so a chunked bf16
  ReduceScatter (3 chunks of 49*128 rows per window) hands each core its
  window's new embeddings at half the collective traffic (the RS/SDMA
  contention otherwise stalls gather descriptor DMA). acc accumulates in
  fp32 in SBUF; final AllGather + windowed gathers of user/item rows +
  dot products produce gamma.
"""
import os
import numpy as np
import ml_dtypes

BF = ml_dtypes.bfloat16
LAST_RESULTS = None  # BassKernelResults of the most recent run (for test.py)

N_USERS = 100000
N_ITEMS = 50000
N = N_USERS + N_ITEMS
D = 64
NC = 8
WIN = 18816             # 147 * 128 nodes per core window
N_PAD = NC * WIN        # 150528
NBLOCKS = N_PAD // 128  # 1176
NCHUNK = 7              # must divide 147 so QROWS is a tile multiple
QROWS = WIN // NCHUNK   # 2688 = 21*128
CHROWS = NC * QROWS     # 21504 rows per P' chunk
BPC = NBLOCKS // NCHUNK # 168 blocks per chunk (multiple of 8)
ROWB = 128              # table row elems (256B)
NB = 48                 # tiles per gather call
CALL = NB * 128         # slots per call
N_LAYERS = 3


def _chunk_of_block(b):
    return ((b * 128) % WIN) // QROWS


def _block_order():
    return sorted(range(NBLOCKS), key=lambda b: (_chunk_of_block(b), b))


def _prep_layer(src, dst, val, dst_mask=None):
    """Build the packed slot stream for one layer.

    Returns (sched, per_core, S_pad) where sched = [(block, m_b, off_b)]
    in chunk-major order with m_b = max over cores of the block's edge
    count rounded up to 32 (PE base-partition granularity), offsets
    nudged so no block starts at partition 96 (illegal matmul base), and
    per_core = [(idx16, scol, val)] packed streams of length S_pad.
    """
    if dst_mask is not None:
        sel = dst_mask[dst]
        src, dst, val = src[sel], dst[sel], val[sel]
    core = src // WIN
    blk = dst // 128

    counts = np.zeros((NC, NBLOCKS), dtype=np.int64)
    np.add.at(counts, (core, blk), 1)
    # 128-quantized block sizes: every matmul fragment is a full-K tile
    # (base partition 0), the only shape proven safe on HW.
    m = (np.maximum(1, counts.max(axis=0)) + 127) // 128 * 128

    order = _block_order()
    sched = []
    offs = {}
    off = 0
    for b in order:
        sched.append((b, int(m[b]), off))
        offs[b] = off
        off += int(m[b])
    S_raw = off
    S_pad = -(-S_raw // CALL) * CALL

    ordk = np.lexsort((dst, blk, core))
    src_s, dst_s, val_s, core_s, blk_s = (
        src[ordk], dst[ordk], val[ordk], core[ordk], blk[ordk])
    key = core_s * NBLOCKS + blk_s
    per_core = []
    for c in range(NC):
        idx16 = np.zeros(S_pad, dtype=np.int16)
        scol = np.zeros(S_pad, dtype=np.float32)
        v = np.zeros(S_pad, dtype=np.float32)
        lo = np.searchsorted(key, c * NBLOCKS)
        hi = np.searchsorted(key, (c + 1) * NBLOCKS)
        bsrc, bdst, bval, bblk = (
            src_s[lo:hi], dst_s[lo:hi], val_s[lo:hi], blk_s[lo:hi])
        bounds = np.searchsorted(bblk, np.arange(NBLOCKS + 1))
        for b in range(NBLOCKS):
            a, e = bounds[b], bounds[b + 1]
            if e == a:
                continue
            o = offs[b]
            n = e - a
            idx16[o:o + n] = (bsrc[a:e] - c * WIN).astype(np.int16)
            scol[o:o + n] = (bdst[a:e] - b * 128).astype(np.float32)
            v[o:o + n] = bval[a:e]
        per_core.append((idx16, scol, v))
    return sched, per_core, S_pad


def _legal_ranges(p0, p1):
    """Split a partition range into PE-legal (base, end) pieces.

    Matmul base partition must be in {0, 32, 64}; K <= 32 at base 32 and
    K <= 64 at base 64. Block offsets are 32-aligned and never 96.
    """
    if p0 == 0 or p0 == 64:
        return [(p0, p1)]
    assert p0 == 32
    if p1 <= 64:
        return [(32, p1)]
    return [(32, 64), (64, p1)]


def _layer_frags(sched, S_pad):
    """Per-call matmul fragments.

    frag = (tile, p0, p1, bpos, first, last). Tiles are 128-slot windows
    of the packed stream; a block's slots [off, off+m) may span several
    tiles and share boundary tiles with neighbors.
    """
    n_calls = S_pad // CALL
    frags_by_call = [[] for _ in range(n_calls)]
    for bpos, (b, mb, off) in enumerate(sched):
        lo, hi = off, off + mb
        t0, t1 = lo // 128, (hi - 1) // 128
        pieces = []
        for t in range(t0, t1 + 1):
            p0 = max(0, lo - t * 128)
            p1 = min(128, hi - t * 128)
            for q0, q1 in _legal_ranges(p0, p1):
                pieces.append((t, q0, q1))
        for i, (t, q0, q1) in enumerate(pieces):
            frags_by_call[t // NB].append(
                (t % NB, q0, q1, bpos, i == 0, i == len(pieces) - 1))
    return frags_by_call


def _wrap_idx(idx_flat):
    S = idx_flat.shape[0]
    w = idx_flat.reshape(S // 16, 16).T
    return np.tile(w, (8, 1)).copy()


def _wrap_slots(a_flat):
    S = a_flat.shape[0]
    return a_flat.reshape(S // 128, 128).T.copy()


def _prep_final(users, items):
    B = users.shape[0]
    pcn = B // NC
    u = users.astype(np.int64)
    it = items.astype(np.int64) + N_USERS
    uw = u // WIN
    iw = it // WIN
    combos = sorted({(int(a), int(b)) for a, b in zip(uw, iw)})
    gmax = {}
    for c in range(NC):
        lo, hi = c * pcn, (c + 1) * pcn
        for cu, ci in combos:
            mm = int(((uw[lo:hi] == cu) & (iw[lo:hi] == ci)).sum())
            gmax[(cu, ci)] = max(gmax.get((cu, ci), 0), mm)
    combos_sizes = [(k, (gmax[k] + 127) // 128 * 128) for k in combos]
    pad_total = sum(s for _, s in combos_sizes)
    plans = []
    for c in range(NC):
        lo, hi = c * pcn, (c + 1) * pcn
        slots_pair = np.full(pad_total, -1, dtype=np.int64)
        uidx = np.zeros(pad_total, dtype=np.int16)
        iidx = np.zeros(pad_total, dtype=np.int16)
        usub, isub = [], []
        off = 0
        for (cu, ci), size in combos_sizes:
            sel = np.nonzero((uw[lo:hi] == cu) & (iw[lo:hi] == ci))[0]
            n = sel.shape[0]
            slots_pair[off:off + n] = lo + sel
            uidx[off:off + n] = (u[lo + sel] - cu * WIN).astype(np.int16)
            iidx[off:off + n] = (it[lo + sel] - ci * WIN).astype(np.int16)
            usub.append((cu, off, off + size))
            isub.append((ci, off, off + size))
            off += size
        plans.append({"slots_pair": slots_pair, "uidx": uidx, "iidx": iidx,
                      "usub": usub, "isub": isub, "ntot": pad_total})
    return plans, combos_sizes, pad_total


def _build_program(scheds, S_pads, fin_subs, FS):
    """Build + compile the shared 8-core program.

    scheds/S_pads: per-layer schedules (layer 0 and 1 share stream
    tensors; layer 2 has its own frontier-restricted stream).
    """
    import concourse.bacc as bacc
    import concourse.tile as tile
    from concourse import mybir

    FT = FS // 128
    nc = bacc.Bacc("TRN2", target_bir_lowering=False, debug=False,
                   num_devices=NC)
    dt = mybir.dt

    t0_in = nc.dram_tensor("t0", [WIN, ROWB], dt.bfloat16, kind="ExternalInput")
    x0_in = nc.dram_tensor("x0", [WIN, D], dt.float32, kind="ExternalInput")
    T01 = S_pads[0] // 128
    T2 = S_pads[2] // 128
    idx01_in = nc.dram_tensor("idx01", [128, T01 * 8], dt.int16,
                              kind="ExternalInput")
    scol01_in = nc.dram_tensor("scol01", [128, T01], dt.bfloat16,
                               kind="ExternalInput")
    val01_in = nc.dram_tensor("val01", [128, T01], dt.bfloat16,
                              kind="ExternalInput")
    idx2_in = nc.dram_tensor("idx2", [128, T2 * 8], dt.int16,
                             kind="ExternalInput")
    scol2_in = nc.dram_tensor("scol2", [128, T2], dt.bfloat16,
                              kind="ExternalInput")
    val2_in = nc.dram_tensor("val2", [128, T2], dt.bfloat16,
                             kind="ExternalInput")
    iota_in = nc.dram_tensor("iota", [128, 128], dt.bfloat16,
                             kind="ExternalInput")
    fu_in = nc.dram_tensor("fuidx", [128, FS // 16], dt.int16,
                           kind="ExternalInput")
    fi_in = nc.dram_tensor("fiidx", [128, FS // 16], dt.int16,
                           kind="ExternalInput")
    gamma_out = nc.dram_tensor("gamma", [128, FT], dt.float32,
                               kind="ExternalOutput")

    tp = [nc.dram_tensor(f"tbl{i}", [WIN, ROWB], dt.bfloat16, kind="Internal")
          for i in range(2)]
    read_t = [t0_in, tp[0], tp[1]]
    write_t = [tp[0], tp[1], None]
    lay_in = [(idx01_in, scol01_in, val01_in),
              (idx01_in, scol01_in, val01_in),
              (idx2_in, scol2_in, val2_in)]
    A_tab = nc.dram_tensor("atab", [N_PAD, D], dt.float32, kind="Internal")
    ag_in = nc.dram_tensor("agin", [WIN, D], dt.float32, kind="Internal")
    pp = [[nc.dram_tensor(f"pp_l{l}_q{q}", [CHROWS, D], dt.bfloat16,
                          kind="Internal") for q in range(NCHUNK)]
          for l in range(N_LAYERS)]
    rs = [[nc.dram_tensor(f"rs_l{l}_q{q}", [QROWS, D], dt.bfloat16,
                          kind="Internal")
           for q in range(NCHUNK)] for l in range(N_LAYERS)]

    frags_by_layer = [_layer_frags(scheds[l], S_pads[l])
                      for l in range(N_LAYERS)]

    with tile.TileContext(nc) as tc:
        with tc.tile_pool(name="persist", bufs=1) as pers, \
             tc.tile_pool(name="inb", bufs=6) as ipool, \
             tc.tile_pool(name="gbuf", bufs=4) as gpool, \
             tc.tile_pool(name="sbuf2", bufs=2) as spool, \
             tc.tile_pool(name="psum", bufs=3, space="PSUM") as ppool:

            iota_t = pers.tile([128, 128], dt.bfloat16)
            nc.sync.dma_start(iota_t[:], iota_in[:])
            acc_t = pers.tile([128, NCHUNK, QROWS // 128, D], dt.float32)
            nc.sync.dma_start(
                acc_t[:],
                x0_in.ap().rearrange("(q p t) d -> p q t d",
                                     q=NCHUNK, p=128, t=QROWS // 128))
            fu_t = pers.tile([128, FS // 16], dt.int16)
            nc.sync.dma_start(fu_t[:], fu_in[:])
            fi_t = pers.tile([128, FS // 16], dt.int16)
            nc.sync.dma_start(fi_t[:], fi_in[:])

            def emit_rs(layer, q):
                nc.gpsimd.collective_compute(
                    "ReduceScatter", mybir.AluOpType.add,
                    replica_groups=[list(range(NC))],
                    ins=[pp[layer][q].ap()],
                    outs=[rs[layer][q].ap()])
                rsb = spool.tile([128, QROWS // 128, D],
                                 dt.bfloat16, tag="rsb")
                nc.sync.dma_start(
                    rsb[:],
                    rs[layer][q].ap().rearrange(
                        "(p t) d -> p t d", p=128, t=QROWS // 128))
                if write_t[layer] is not None:
                    nc.sync.dma_start(
                        write_t[layer][q * QROWS:(q + 1) * QROWS,
                                       0:D].rearrange(
                            "(p t) d -> p t d", p=128, t=QROWS // 128),
                        rsb[:])
                nc.vector.tensor_tensor(
                    acc_t[:, q, :, :], acc_t[:, q, :, :], rsb[:],
                    mybir.AluOpType.add)
                if layer == N_LAYERS - 1:
                    nc.sync.dma_start(
                        ag_in.ap().rearrange(
                            "(q p t) d -> p q t d",
                            q=NCHUNK, p=128,
                            t=QROWS // 128)[:, q:q + 1, :, :],
                        acc_t[:, q:q + 1, :, :])

            RS_DELAY = 2
            for layer in range(N_LAYERS):
                tbl = read_t[layer]
                idx_in, scol_in, val_in = lay_in[layer]
                frags_by_call = frags_by_layer[layer]
                n_calls = S_pads[layer] // CALL
                # call index where each chunk's last pp write is emitted
                last_call = {}
                for cj, frs in enumerate(frags_by_call):
                    for (_t, _p0, _p1, bpos, _f, lst) in frs:
                        if lst and (bpos + 1) % BPC == 0:
                            last_call[bpos // BPC] = cj
                rs_sched = {}
                rs_tail = []
                for q in range(NCHUNK):
                    tgt = last_call[q] + RS_DELAY
                    if tgt < n_calls:
                        rs_sched.setdefault(tgt, []).append(q)
                    else:
                        rs_tail.append(q)
                psum_t = None
                stage_t = None
                for ci in range(n_calls):
                    for q in rs_sched.get(ci, []):
                        emit_rs(layer, q)
                    idx_t = ipool.tile([128, NB * 8], dt.int16, tag="idx")
                    nc.sync.dma_start(
                        idx_t[:], idx_in[:, ci * NB * 8:(ci + 1) * NB * 8])
                    scol_t = ipool.tile([128, NB], dt.bfloat16, tag="scol")
                    nc.sync.dma_start(
                        scol_t[:], scol_in[:, ci * NB:(ci + 1) * NB])
                    val_t = ipool.tile([128, NB], dt.bfloat16, tag="val")
                    nc.sync.dma_start(
                        val_t[:], val_in[:, ci * NB:(ci + 1) * NB])
                    g_t = gpool.tile([128, NB, ROWB], dt.bfloat16, tag="g")
                    # ucode limit: 1024 idxs per dma_gather call
                    for k in range(NB // 8):
                        nc.gpsimd.dma_gather(
                            out_ap=g_t[:, k * 8:(k + 1) * 8, :],
                            in_ap=tbl[:],
                            idxs_ap=idx_t[:, k * 64:(k + 1) * 64],
                            num_idxs=1024, num_idxs_reg=1024,
                            elem_size=ROWB)
                    seq_t = spool.tile([128, NB, 128], dt.bfloat16, tag="seq")
                    nc.vector.tensor_tensor(
                        seq_t[:],
                        scol_t[:].unsqueeze(-1).broadcast_to([128, NB, 128]),
                        iota_t[:].unsqueeze(1).broadcast_to([128, NB, 128]),
                        mybir.AluOpType.is_equal)
                    s_t = spool.tile([128, NB, 128], dt.bfloat16, tag="s")
                    nc.vector.tensor_tensor(
                        s_t[:],
                        seq_t[:],
                        val_t[:].unsqueeze(-1).broadcast_to([128, NB, 128]),
                        mybir.AluOpType.mult)

                    for (t, p0, p1, bpos, first, last) in frags_by_call[ci]:
                        if first and bpos % 4 == 0:
                            psum_t = ppool.tile([128, 4, D], dt.float32)
                        nc.tensor.matmul(
                            psum_t[:, bpos % 4, :],
                            s_t[p0:p1, t, :], g_t[p0:p1, t, 0:D],
                            start=first, stop=last)
                        if not last:
                            continue
                        # block bpos complete
                        if bpos % 8 == 0:
                            stage_t = spool.tile([128, 8, D], dt.bfloat16,
                                                 tag="stage")
                        if bpos % 4 == 3:
                            nc.scalar.activation(
                                stage_t[:, (bpos % 8) - 3:(bpos % 8) + 1, :],
                                psum_t[:],
                                mybir.ActivationFunctionType.Copy)
                        if bpos % 8 == 7:
                            q = bpos // BPC
                            r0 = (bpos % BPC - 7) * 128
                            nc.sync.dma_start(
                                pp[layer][q][r0:r0 + 1024, :].rearrange(
                                    "(j p) d -> p j d", p=128, j=8),
                                stage_t[:])
                for q in rs_tail:
                    emit_rs(layer, q)

            # final: AllGather acc (scaling folded into gamma) -> gathers -> dots
            nc.gpsimd.collective_compute(
                "AllGather", mybir.AluOpType.bypass,
                replica_groups=[list(range(NC))],
                ins=[ag_in.ap()], outs=[A_tab.ap()])

            ug_t = pers.tile([128, FT, D], dt.float32)
            ig_t = pers.tile([128, FT, D], dt.float32)
            for subs, idxt, outt in ((fin_subs[0], fu_t, ug_t),
                                     (fin_subs[1], fi_t, ig_t)):
                for w, lo, hi in subs:
                    for a in range(lo, hi, 1024):
                        n = min(1024, hi - a)
                        nc.gpsimd.dma_gather(
                            out_ap=outt[:, a // 128:(a + n) // 128, :],
                            in_ap=A_tab[w * WIN:(w + 1) * WIN, :],
                            idxs_ap=idxt[:, a // 16:(a + n) // 16],
                            num_idxs=n, num_idxs_reg=n,
                            elem_size=D)
            prod_t = pers.tile([128, FT, D], dt.float32)
            nc.vector.tensor_mul(prod_t[:], ug_t[:], ig_t[:])
            gam_t = pers.tile([128, FT], dt.float32)
            nc.vector.tensor_reduce(
                gam_t[:].unsqueeze(-1), prod_t[:],
                axis=mybir.AxisListType.X, op=mybir.AluOpType.add)
            nc.vector.tensor_scalar_mul(gam_t[:], gam_t[:], 1.0 / 16.0)
            nc.sync.dma_start(gamma_out[:], gam_t[:])

    nc.compile()
    return nc


def kernel(**inputs):
    from concourse import bass_utils

    users = np.asarray(inputs["users"])
    items = np.asarray(inputs["items"])
    edge_src = np.asarray(inputs["edge_src"]).astype(np.int64)
    edge_dst = np.asarray(inputs["edge_dst"]).astype(np.int64)
    edge_val = np.asarray(inputs["edge_val"], dtype=np.float32)
    user_emb = np.asarray(inputs["user_emb"], dtype=np.float32)
    item_emb = np.asarray(inputs["item_emb"], dtype=np.float32)

    F = np.union1d(np.unique(users.astype(np.int64)),
                   np.unique(items.astype(np.int64) + N_USERS))
    fmask = np.zeros(N_PAD, dtype=bool)
    fmask[F] = True

    sched01, per_core01, S01 = _prep_layer(edge_src, edge_dst, edge_val)
    sched2, per_core2, S2 = _prep_layer(edge_src, edge_dst, edge_val,
                                        dst_mask=fmask)
    scheds = [sched01, sched01, sched2]
    S_pads = [S01, S01, S2]

    plans, combos_sizes, FS = _prep_final(users, items)
    fin_subs = (plans[0]["usub"], plans[0]["isub"])

    nc = _build_program(scheds, S_pads, fin_subs, FS)

    x0 = np.zeros((N_PAD, D), dtype=np.float32)
    x0[:N_USERS] = user_emb
    x0[N_USERS:N] = item_emb
    iota_img = np.tile(np.arange(128, dtype=np.float32).astype(BF)[None, :],
                       (128, 1)).copy()

    in_maps = []
    for c in range(NC):
        idx01, scol01, val01 = per_core01[c]
        idx2, scol2, val2 = per_core2[c]
        t0 = np.zeros((WIN, ROWB), dtype=BF)
        t0[:, :D] = x0[c * WIN:(c + 1) * WIN].astype(BF)
        pl = plans[c]
        in_maps.append({
            "t0": t0,
            "x0": x0[c * WIN:(c + 1) * WIN].copy(),
            "idx01": _wrap_idx(idx01),
            "scol01": _wrap_slots(scol01.astype(BF)),
            "val01": _wrap_slots(val01.astype(BF)),
            "idx2": _wrap_idx(idx2),
            "scol2": _wrap_slots(scol2.astype(BF)),
            "val2": _wrap_slots(val2.astype(BF)),
            "iota": iota_img,
            "fuidx": _wrap_idx(pl["uidx"]),
            "fiidx": _wrap_idx(pl["iidx"]),
        })

    res = bass_utils.run_bass_kernel_spmd(
        nc, in_maps, core_ids=list(range(NC)),
        trace=bool(os.environ.get("KERNEL_TRACE")))
    global LAST_RESULTS
    LAST_RESULTS = res

    gamma = np.zeros(users.shape[0], dtype=np.float32)
    for c in range(NC):
        img = res.results[c]["gamma"]          # [128, FT]
        flat = img.T.reshape(-1)               # slot s = 128*t + p
        pl = plans[c]
        sel = pl["slots_pair"] >= 0
        gamma[pl["slots_pair"][sel]] = flat[sel]
    return gamma


# revision 11
# speedup vs baseline: 1.0321x; 1.0002x over previous
"""LightGCN on 8 Trainium2 NeuronCores (Bass/Tile).

Scheme (src-sharded, ReduceScatter), v4:
- Node table padded to N_PAD = 8*18816 rows, stored per core as a bf16
  [WIN, 128] window table (cols 0:64 = features, 64:128 pad so rows are
  256B for dma_gather).
- Edge (src, dst, val) is processed by core src // WIN. Per core, edges
  are bucketed into 1176 dst-blocks of 128 dsts laid out chunk-major in a
  slot stream; block b occupies ceil128(max_core(count(c,b))) slots.
  (Sub-128-K matmul fragments at base partitions 32/64 crash on HW, so
  blocks stay 128-quantized; the frag machinery still supports packing.)
- Layer 3 frontier sparsification: gamma only reads acc rows for the
  batch's distinct users/items (F, ~29K rows), so layer 3 processes only
  edges with dst in F (~19% of edges), cutting its gather descriptor
  load (the bottleneck: ~9ns/row on the serial GpSimd Q7) by ~70%.
  Non-F output rows become zero, which is harmless in acc.
- Per call of NB tiles: dma_gather x[src] rows (bf16), build
  S[slot, dstcol] = val * (scol == iota) with 2 DVE ops, TensorEngine
  matmul fragments S^T @ X accumulate each block's [128, 64] partial sums
  in PSUM.
- Partials go bf16 to a permuted DRAM table P' so a chunked bf16
  ReduceScatter (3 chunks of 49*128 rows per window) hands each core its
  window's new embeddings at half the collective traffic (the RS/SDMA
  contention otherwise stalls gather descriptor DMA). acc accumulates in
  fp32 in SBUF; final AllGather + windowed gathers of user/item rows +
  dot products produce gamma.
"""
import os
import numpy as np
import ml_dtypes

BF = ml_dtypes.bfloat16
LAST_RESULTS = None  # BassKernelResults of the most recent run (for test.py)

N_USERS = 100000
N_ITEMS = 50000
N = N_USERS + N_ITEMS
D = 64
NC = 8
WIN = 18816             # 147 * 128 nodes per core window
N_PAD = NC * WIN        # 150528
NBLOCKS = N_PAD // 128  # 1176
NCHUNK = 7              # must divide 147 so QROWS is a tile multiple
QROWS = WIN // NCHUNK   # 2688 = 21*128
CHROWS = NC * QROWS     # 21504 rows per P' chunk
BPC = NBLOCKS // NCHUNK # 168 blocks per chunk (multiple of 8)
ROWB = 128              # table row elems (256B)
NB = 48                 # tiles per gather call
CALL = NB * 128         # slots per call
N_LAYERS = 3


def _chunk_of_block(b):
    return ((b * 128) % WIN) // QROWS


def _block_order():
    return sorted(range(NBLOCKS), key=lambda b: (_chunk_of_block(b), b))


def _prep_layer(src, dst, val, dst_mask=None):
    """Build the packed slot stream for one layer.

    Returns (sched, per_core, S_pad) where sched = [(block, m_b, off_b)]
    in chunk-major order with m_b = max over cores of the block's edge
    count rounded up to 32 (PE base-partition granularity), offsets
    nudged so no block starts at partition 96 (illegal matmul base), and
    per_core = [(idx16, scol, val)] packed streams of length S_pad.
    """
    if dst_mask is not None:
        sel = dst_mask[dst]
        src, dst, val = src[sel], dst[sel], val[sel]
    core = src // WIN
    blk = dst // 128

    counts = np.zeros((NC, NBLOCKS), dtype=np.int64)
    np.add.at(counts, (core, blk), 1)
    # 128-quantized block sizes: every matmul fragment is a full-K tile
    # (base partition 0), the only shape proven safe on HW.
    m = (np.maximum(1, counts.max(axis=0)) + 127) // 128 * 128

    order = _block_order()
    sched = []
    offs = {}
    off = 0
    for b in order:
        sched.append((b, int(m[b]), off))
        offs[b] = off
        off += int(m[b])
    S_raw = off
    S_pad = -(-S_raw // CALL) * CALL

    ordk = np.lexsort((dst, blk, core))
    src_s, dst_s, val_s, core_s, blk_s = (
        src[ordk], dst[ordk], val[ordk], core[ordk], blk[ordk])
    key = core_s * NBLOCKS + blk_s
    per_core = []
    for c in range(NC):
        idx16 = np.zeros(S_pad, dtype=np.int16)
        scol = np.zeros(S_pad, dtype=np.float32)
        v = np.zeros(S_pad, dtype=np.float32)
        lo = np.searchsorted(key, c * NBLOCKS)
        hi = np.searchsorted(key, (c + 1) * NBLOCKS)
        bsrc, bdst, bval, bblk = (
            src_s[lo:hi], dst_s[lo:hi], val_s[lo:hi], blk_s[lo:hi])
        bounds = np.searchsorted(bblk, np.arange(NBLOCKS + 1))
        for b in range(NBLOCKS):
            a, e = bounds[b], bounds[b + 1]
            if e == a:
                continue
            o = offs[b]
            n = e - a
            idx16[o:o + n] = (bsrc[a:e] - c * WIN).astype(np.int16)
            scol[o:o + n] = (bdst[a:e] - b * 128).astype(np.float32)
            v[o:o + n] = bval[a:e]
        per_core.append((idx16, scol, v))
    return sched, per_core, S_pad


def _legal_ranges(p0, p1):
    """Split a partition range into PE-legal (base, end) pieces.

    Matmul base partition must be in {0, 32, 64}; K <= 32 at base 32 and
    K <= 64 at base 64. Block offsets are 32-aligned and never 96.
    """
    if p0 == 0 or p0 == 64:
        return [(p0, p1)]
    assert p0 == 32
    if p1 <= 64:
        return [(32, p1)]
    return [(32, 64), (64, p1)]


def _layer_frags(sched, S_pad):
    """Per-call matmul fragments.

    frag = (tile, p0, p1, bpos, first, last). Tiles are 128-slot windows
    of the packed stream; a block's slots [off, off+m) may span several
    tiles and share boundary tiles with neighbors.
    """
    n_calls = S_pad // CALL
    frags_by_call = [[] for _ in range(n_calls)]
    for bpos, (b, mb, off) in enumerate(sched):
        lo, hi = off, off + mb
        t0, t1 = lo // 128, (hi - 1) // 128
        pieces = []
        for t in range(t0, t1 + 1):
            p0 = max(0, lo - t * 128)
            p1 = min(128, hi - t * 128)
            for q0, q1 in _legal_ranges(p0, p1):
                pieces.append((t, q0, q1))
        for i, (t, q0, q1) in enumerate(pieces):
            frags_by_call[t // NB].append(
                (t % NB, q0, q1, bpos, i == 0, i == len(pieces) - 1))
    return frags_by_call


def _wrap_idx(idx_flat):
    S = idx_flat.shape[0]
    w = idx_flat.reshape(S // 16, 16).T
    return np.tile(w, (8, 1)).copy()


def _wrap_slots(a_flat):
    S = a_flat.shape[0]
    return a_flat.reshape(S // 128, 128).T.copy()


def _prep_final(users, items):
    B = users.shape[0]
    pcn = B // NC
    u = users.astype(np.int64)
    it = items.astype(np.int64) + N_USERS
    uw = u // WIN
    iw = it // WIN
    combos = sorted({(int(a), int(b)) for a, b in zip(uw, iw)})
    gmax = {}
    for c in range(NC):
        lo, hi = c * pcn, (c + 1) * pcn
        for cu, ci in combos:
            mm = int(((uw[lo:hi] == cu) & (iw[lo:hi] == ci)).sum())
            gmax[(cu, ci)] = max(gmax.get((cu, ci), 0), mm)
    combos_sizes = [(k, (gmax[k] + 127) // 128 * 128) for k in combos]
    pad_total = sum(s for _, s in combos_sizes)
    plans = []
    for c in range(NC):
        lo, hi = c * pcn, (c + 1) * pcn
        slots_pair = np.full(pad_total, -1, dtype=np.int64)
        uidx = np.zeros(pad_total, dtype=np.int16)
        iidx = np.zeros(pad_total, dtype=np.int16)
        usub, isub = [], []
        off = 0
        for (cu, ci), size in combos_sizes:
            sel = np.nonzero((uw[lo:hi] == cu) & (iw[lo:hi] == ci))[0]
            n = sel.shape[0]
            slots_pair[off:off + n] = lo + sel
            uidx[off:off + n] = (u[lo + sel] - cu * WIN).astype(np.int16)
            iidx[off:off + n] = (it[lo + sel] - ci * WIN).astype(np.int16)
            usub.append((cu, off, off + size))
            isub.append((ci, off, off + size))
            off += size
        plans.append({"slots_pair": slots_pair, "uidx": uidx, "iidx": iidx,
                      "usub": usub, "isub": isub, "ntot": pad_total})
    return plans, combos_sizes, pad_total


def _build_program(scheds, S_pads, fin_subs, FS):
    """Build + compile the shared 8-core program.

    scheds/S_pads: per-layer schedules (layer 0 and 1 share stream
    tensors; layer 2 has its own frontier-restricted stream).
    """
    import concourse.bacc as bacc
    import concourse.tile as tile
    from concourse import mybir

    FT = FS // 128
    nc = bacc.Bacc("TRN2", target_bir_lowering=False, debug=False,
                   num_devices=NC)
    dt = mybir.dt

    t0_in = nc.dram_tensor("t0", [WIN, ROWB], dt.bfloat16, kind="ExternalInput")
    x0_in = nc.dram_tensor("x0", [WIN, D], dt.float32, kind="ExternalInput")
    T01 = S_pads[0] // 128
    T2 = S_pads[2] // 128
    idx01_in = nc.dram_tensor("idx01", [128, T01 * 8], dt.int16,
                              kind="ExternalInput")
    scol01_in = nc.dram_tensor("scol01", [128, T01], dt.bfloat16,
                               kind="ExternalInput")
    val01_in = nc.dram_tensor("val01", [128, T01], dt.bfloat16,
                              kind="ExternalInput")
    idx2_in = nc.dram_tensor("idx2", [128, T2 * 8], dt.int16,
                             kind="ExternalInput")
    scol2_in = nc.dram_tensor("scol2", [128, T2], dt.bfloat16,
                              kind="ExternalInput")
    val2_in = nc.dram_tensor("val2", [128, T2], dt.bfloat16,
                             kind="ExternalInput")
    iota_in = nc.dram_tensor("iota", [128, 128], dt.bfloat16,
                             kind="ExternalInput")
    fu_in = nc.dram_tensor("fuidx", [128, FS // 16], dt.int16,
                           kind="ExternalInput")
    fi_in = nc.dram_tensor("fiidx", [128, FS // 16], dt.int16,
                           kind="ExternalInput")
    gamma_out = nc.dram_tensor("gamma", [128, FT], dt.float32,
                               kind="ExternalOutput")

    tp = [nc.dram_tensor(f"tbl{i}", [WIN, ROWB], dt.bfloat16, kind="Internal")
          for i in range(2)]
    read_t = [t0_in, tp[0], tp[1]]
    write_t = [tp[0], tp[1], None]
    lay_in = [(idx01_in, scol01_in, val01_in),
              (idx01_in, scol01_in, val01_in),
              (idx2_in, scol2_in, val2_in)]
    A_tab = nc.dram_tensor("atab", [N_PAD, D], dt.float32, kind="Internal")
    ag_in = nc.dram_tensor("agin", [WIN, D], dt.float32, kind="Internal")
    pp = [[nc.dram_tensor(f"pp_l{l}_q{q}", [CHROWS, D], dt.bfloat16,
                          kind="Internal") for q in range(NCHUNK)]
          for l in range(N_LAYERS)]
    rs = [[nc.dram_tensor(f"rs_l{l}_q{q}", [QROWS, D], dt.bfloat16,
                          kind="Internal")
           for q in range(NCHUNK)] for l in range(N_LAYERS)]

    frags_by_layer = [_layer_frags(scheds[l], S_pads[l])
                      for l in range(N_LAYERS)]

    with tile.TileContext(nc) as tc:
        with tc.tile_pool(name="persist", bufs=1) as pers, \
             tc.tile_pool(name="inb", bufs=6) as ipool, \
             tc.tile_pool(name="gbuf", bufs=4) as gpool, \
             tc.tile_pool(name="sbuf2", bufs=2) as spool, \
             tc.tile_pool(name="rsbp", bufs=8) as rsbpool, \
             tc.tile_pool(name="psum", bufs=3, space="PSUM") as ppool:

            iota_t = pers.tile([128, 128], dt.bfloat16)
            nc.sync.dma_start(iota_t[:], iota_in[:])
            acc_t = pers.tile([128, NCHUNK, QROWS // 128, D], dt.float32)
            nc.sync.dma_start(
                acc_t[:],
                x0_in.ap().rearrange("(q p t) d -> p q t d",
                                     q=NCHUNK, p=128, t=QROWS // 128))
            fu_t = pers.tile([128, FS // 16], dt.int16)
            nc.sync.dma_start(fu_t[:], fu_in[:])
            fi_t = pers.tile([128, FS // 16], dt.int16)
            nc.sync.dma_start(fi_t[:], fi_in[:])

            rsb_tiles = {}

            def emit_rs(layer, q):
                nc.gpsimd.collective_compute(
                    "ReduceScatter", mybir.AluOpType.add,
                    replica_groups=[list(range(NC))],
                    ins=[pp[layer][q].ap()],
                    outs=[rs[layer][q].ap()])
                rsb = rsbpool.tile([128, QROWS // 128, D],
                                   dt.bfloat16, tag="rsb")
                nc.sync.dma_start(
                    rsb[:],
                    rs[layer][q].ap().rearrange(
                        "(p t) d -> p t d", p=128, t=QROWS // 128))
                if write_t[layer] is not None:
                    nc.sync.dma_start(
                        write_t[layer][q * QROWS:(q + 1) * QROWS,
                                       0:D].rearrange(
                            "(p t) d -> p t d", p=128, t=QROWS // 128),
                        rsb[:])
                rsb_tiles[(layer, q)] = rsb

            def emit_acc(layer, q):
                rsb = rsb_tiles.pop((layer, q))
                nc.vector.tensor_tensor(
                    acc_t[:, q, :, :], acc_t[:, q, :, :], rsb[:],
                    mybir.AluOpType.add)
                if layer == N_LAYERS - 1:
                    nc.sync.dma_start(
                        ag_in.ap().rearrange(
                            "(q p t) d -> p q t d",
                            q=NCHUNK, p=128,
                            t=QROWS // 128)[:, q:q + 1, :, :],
                        acc_t[:, q:q + 1, :, :])

            RS_DELAY = 2
            ACC_DELAY = 5
            acc_carry = []
            for layer in range(N_LAYERS):
                tbl = read_t[layer]
                idx_in, scol_in, val_in = lay_in[layer]
                frags_by_call = frags_by_layer[layer]
                n_calls = S_pads[layer] // CALL
                # call index where each chunk's last pp write is emitted
                last_call = {}
                for cj, frs in enumerate(frags_by_call):
                    for (_t, _p0, _p1, bpos, _f, lst) in frs:
                        if lst and (bpos + 1) % BPC == 0:
                            last_call[bpos // BPC] = cj
                rs_sched = {}
                rs_tail = []
                acc_sched = {}
                for q in range(NCHUNK):
                    tgt = last_call[q] + RS_DELAY
                    if tgt < n_calls:
                        rs_sched.setdefault(tgt, []).append(q)
                    else:
                        rs_tail.append(q)
                    tgt2 = last_call[q] + ACC_DELAY
                    if tgt2 < n_calls:
                        acc_sched.setdefault(tgt2, []).append(q)
                    else:
                        acc_carry.append((layer, q))
                psum_t = None
                stage_t = None
                for ci in range(n_calls):
                    if ci == 0 and acc_carry:
                        prev, acc_carry = acc_carry, []
                        for (pl, q) in prev:
                            if pl == layer:
                                acc_carry.append((pl, q))
                            else:
                                emit_acc(pl, q)
                    for q in rs_sched.get(ci, []):
                        emit_rs(layer, q)
                    for q in acc_sched.get(ci, []):
                        emit_acc(layer, q)
                    idx_t = ipool.tile([128, NB * 8], dt.int16, tag="idx")
                    nc.sync.dma_start(
                        idx_t[:], idx_in[:, ci * NB * 8:(ci + 1) * NB * 8])
                    scol_t = ipool.tile([128, NB], dt.bfloat16, tag="scol")
                    nc.sync.dma_start(
                        scol_t[:], scol_in[:, ci * NB:(ci + 1) * NB])
                    val_t = ipool.tile([128, NB], dt.bfloat16, tag="val")
                    nc.sync.dma_start(
                        val_t[:], val_in[:, ci * NB:(ci + 1) * NB])
                    g_t = gpool.tile([128, NB, ROWB], dt.bfloat16, tag="g")
                    # ucode limit: 1024 idxs per dma_gather call
                    for k in range(NB // 8):
                        nc.gpsimd.dma_gather(
                            out_ap=g_t[:, k * 8:(k + 1) * 8, :],
                            in_ap=tbl[:],
                            idxs_ap=idx_t[:, k * 64:(k + 1) * 64],
                            num_idxs=1024, num_idxs_reg=1024,
                            elem_size=ROWB)
                    seq_t = spool.tile([128, NB, 128], dt.bfloat16, tag="seq")
                    nc.vector.tensor_tensor(
                        seq_t[:],
                        scol_t[:].unsqueeze(-1).broadcast_to([128, NB, 128]),
                        iota_t[:].unsqueeze(1).broadcast_to([128, NB, 128]),
                        mybir.AluOpType.is_equal)
                    s_t = spool.tile([128, NB, 128], dt.bfloat16, tag="s")
                    nc.vector.tensor_tensor(
                        s_t[:],
                        seq_t[:],
                        val_t[:].unsqueeze(-1).broadcast_to([128, NB, 128]),
                        mybir.AluOpType.mult)

                    for (t, p0, p1, bpos, first, last) in frags_by_call[ci]:
                        if first and bpos % 4 == 0:
                            psum_t = ppool.tile([128, 4, D], dt.float32)
                        nc.tensor.matmul(
                            psum_t[:, bpos % 4, :],
                            s_t[p0:p1, t, :], g_t[p0:p1, t, 0:D],
                            start=first, stop=last)
                        if not last:
                            continue
                        # block bpos complete
                        if bpos % 8 == 0:
                            stage_t = spool.tile([128, 8, D], dt.bfloat16,
                                                 tag="stage")
                        if bpos % 4 == 3:
                            nc.scalar.activation(
                                stage_t[:, (bpos % 8) - 3:(bpos % 8) + 1, :],
                                psum_t[:],
                                mybir.ActivationFunctionType.Copy)
                        if bpos % 8 == 7:
                            q = bpos // BPC
                            r0 = (bpos % BPC - 7) * 128
                            nc.sync.dma_start(
                                pp[layer][q][r0:r0 + 1024, :].rearrange(
                                    "(j p) d -> p j d", p=128, j=8),
                                stage_t[:])
                for q in rs_tail:
                    emit_rs(layer, q)
                if layer == N_LAYERS - 1:
                    for (pl, q) in acc_carry:
                        emit_acc(pl, q)
                    acc_carry = []

            # final: AllGather acc (scaling folded into gamma) -> gathers -> dots
            nc.gpsimd.collective_compute(
                "AllGather", mybir.AluOpType.bypass,
                replica_groups=[list(range(NC))],
                ins=[ag_in.ap()], outs=[A_tab.ap()])

            ug_t = pers.tile([128, FT, D], dt.float32)
            ig_t = pers.tile([128, FT, D], dt.float32)
            for subs, idxt, outt in ((fin_subs[0], fu_t, ug_t),
                                     (fin_subs[1], fi_t, ig_t)):
                for w, lo, hi in subs:
                    for a in range(lo, hi, 1024):
                        n = min(1024, hi - a)
                        nc.gpsimd.dma_gather(
                            out_ap=outt[:, a // 128:(a + n) // 128, :],
                            in_ap=A_tab[w * WIN:(w + 1) * WIN, :],
                            idxs_ap=idxt[:, a // 16:(a + n) // 16],
                            num_idxs=n, num_idxs_reg=n,
                            elem_size=D)
            prod_t = pers.tile([128, FT, D], dt.float32)
            nc.vector.tensor_mul(prod_t[:], ug_t[:], ig_t[:])
            gam_t = pers.tile([128, FT], dt.float32)
            nc.vector.tensor_reduce(
                gam_t[:].unsqueeze(-1), prod_t[:],
                axis=mybir.AxisListType.X, op=mybir.AluOpType.add)
            nc.vector.tensor_scalar_mul(gam_t[:], gam_t[:], 1.0 / 16.0)
            nc.sync.dma_start(gamma_out[:], gam_t[:])

    nc.compile()
    return nc


def kernel(**inputs):
    from concourse import bass_utils

    users = np.asarray(inputs["users"])
    items = np.asarray(inputs["items"])
    edge_src = np.asarray(inputs["edge_src"]).astype(np.int64)
    edge_dst = np.asarray(inputs["edge_dst"]).astype(np.int64)
    edge_val = np.asarray(inputs["edge_val"], dtype=np.float32)
    user_emb = np.asarray(inputs["user_emb"], dtype=np.float32)
    item_emb = np.asarray(inputs["item_emb"], dtype=np.float32)

    F = np.union1d(np.unique(users.astype(np.int64)),
                   np.unique(items.astype(np.int64) + N_USERS))
    fmask = np.zeros(N_PAD, dtype=bool)
    fmask[F] = True

    sched01, per_core01, S01 = _prep_layer(edge_src, edge_dst, edge_val)
    sched2, per_core2, S2 = _prep_layer(edge_src, edge_dst, edge_val,
                                        dst_mask=fmask)
    scheds = [sched01, sched01, sched2]
    S_pads = [S01, S01, S2]

    plans, combos_sizes, FS = _prep_final(users, items)
    fin_subs = (plans[0]["usub"], plans[0]["isub"])

    nc = _build_program(scheds, S_pads, fin_subs, FS)

    x0 = np.zeros((N_PAD, D), dtype=np.float32)
    x0[:N_USERS] = user_emb
    x0[N_USERS:N] = item_emb
    iota_img = np.tile(np.arange(128, dtype=np.float32).astype(BF)[None, :],
                       (128, 1)).copy()

    in_maps = []
    for c in range(NC):
        idx01, scol01, val01 = per_core01[c]
        idx2, scol2, val2 = per_core2[c]
        t0 = np.zeros((WIN, ROWB), dtype=BF)
        t0[:, :D] = x0[c * WIN:(c + 1) * WIN].astype(BF)
        pl = plans[c]
        in_maps.append({
            "t0": t0,
            "x0": x0[c * WIN:(c + 1) * WIN].copy(),
            "idx01": _wrap_idx(idx01),
            "scol01": _wrap_slots(scol01.astype(BF)),
            "val01": _wrap_slots(val01.astype(BF)),
            "idx2": _wrap_idx(idx2),
            "scol2": _wrap_slots(scol2.astype(BF)),
            "val2": _wrap_slots(val2.astype(BF)),
            "iota": iota_img,
            "fuidx": _wrap_idx(pl["uidx"]),
            "fiidx": _wrap_idx(pl["iidx"]),
        })

    res = bass_utils.run_bass_kernel_spmd(
        nc, in_maps, core_ids=list(range(NC)),
        trace=bool(os.environ.get("KERNEL_TRACE")))
    global LAST_RESULTS
    LAST_RESULTS = res

    gamma = np.zeros(users.shape[0], dtype=np.float32)
    for c in range(NC):
        img = res.results[c]["gamma"]          # [128, FT]
        flat = img.T.reshape(-1)               # slot s = 128*t + p
        pl = plans[c]
        sel = pl["slots_pair"] >= 0
        gamma[pl["slots_pair"][sel]] = flat[sel]
    return gamma
